# revision 10
# baseline (speedup 1.0000x reference)
"""Trainium2 Bass kernel for the dual-stream "DifAttention" block — v4.

Partitioning (unchanged): 8 independent (batch, stream) units, one per core,
SPMD, no collectives:
    x-core b: t_qk=x[b], t_v=x[b], t_qo=y[b]
    y-core b: t_qk=y[b], t_v=x[b], t_qo=x[b]

v4 changes vs v3 (388906 -> target <220us):

  S^T = K Q^T      bf16 blockdiag instead of fp8 DoubleRow. For the d=64
                   blockdiag structure a plain bf16 matmul covers the same
                   [128m x 512n] region in the same 512 cycles as the DR-fp8
                   variant (DR's pair slot only carried the K-residual
                   correction, not extra throughput), but q/k now carry only
                   bf16 cast error (~0.4%) instead of fp8 (~6%) -> the error
                   budget frees up for the DVE exp below.
  exp split        ACT does 6 of 8 m-tiles per (head,att) via native EXP;
                   DVE does 2 of 8 via a Schraudolph bit-trick:
                   a = bitcast_bf16(int16(s*A + B)) with A = +-EXPSC*log2e*128,
                   B = 128*(127 + c), c = -0.0564 (mean-zero centering so the
                   DVE tiles carry no softmax-mass bias vs the ACT tiles).
                   One DVE tensor_scalar per tile (~1.07us) vs ACT's 1.15us;
                   this moves the pipeline pacer from ACT (9.2us/unit) to the
                   PE (~8us/unit).
  kblk zeros       off-diagonal zero fill via DMA from a DRAM zeros tensor
                   (was ~19us of DVE memsets).
  input DMA        the three 1.5MB x-tensors ride separate engine queues.
  PSUM pools       s-tiles (2 bufs), o-tile (1 buf), filler/proj/transpose
                   (1 buf) in separate pools = 8 banks exactly, so exp
                   draining, AV accumulation and projection fillers don't
                   falsely serialize through one ring.
  projections      split-fp8 DoubleRow, all four 3-term (QO back to full
                   precision; outputs now staged bf16).
"""

import numpy as np
import ml_dtypes

import concourse.bass as bass
import concourse.bacc as bacc
import concourse.tile as tile
from concourse import mybir
from concourse.bass_utils import run_bass_kernel_spmd

P = 128
B, N, C = 4, 1024, 768
H, HD = 12, 64
CT = C // P           # 6 column tiles (= head pairs)
NT = N // P           # 8 sequence tiles
WSCALE = 32.0         # host pre-scale on W (fp8 subnormal avoidance)
EXPSC = 0.125 / (WSCALE * WSCALE)   # = 1/8192, folds the x32*x32 back out

LOG2E = 1.4426950408889634
C_CENTER = -0.056401  # mean-zero Schraudolph centering
EXP_A = EXPSC * LOG2E * 128.0
EXP_B = 128.0 * (127.0 + C_CENTER)
DVE_MTS = (2, 5)      # m-tiles per (head,att) handled by the DVE exp

FP32 = mybir.dt.float32
BF16 = mybir.dt.bfloat16
FP8 = mybir.dt.float8e4
I16 = mybir.dt.int16
DR = mybir.MatmulPerfMode.DoubleRow
EXP = mybir.ActivationFunctionType.Exp
MUL = mybir.AluOpType.mult
ADD = mybir.AluOpType.add


def build_kernel():
    nc = bacc.Bacc("TRN2", target_bir_lowering=False, debug=False,
                   num_devices=8)

    d_qk = nc.dram_tensor("qk8", [C, 2, N], FP8, kind="ExternalInput")
    d_qo = nc.dram_tensor("qo8", [C, 2, N], FP8, kind="ExternalInput")
    d_v = nc.dram_tensor("v8", [C, 2, N], FP8, kind="ExternalInput")
    d_wq = nc.dram_tensor("wq8", [C, 2, C], FP8, kind="ExternalInput")
    d_wk = nc.dram_tensor("wk8", [C, 2, C], FP8, kind="ExternalInput")
    d_wqo = nc.dram_tensor("wqo8", [C, 2, C], FP8, kind="ExternalInput")
    d_wv = nc.dram_tensor("wv8", [C, 2, C], FP8, kind="ExternalInput")
    d_wp = nc.dram_tensor("wp16", [C, C], BF16, kind="ExternalInput")
    d_eye = nc.dram_tensor("eye16", [P, P], BF16, kind="ExternalInput")
    d_zero = nc.dram_tensor("zeros16", [1, 8192], BF16, kind="ExternalInput")
    d_out = nc.dram_tensor("out", [N, C], FP32, kind="ExternalOutput")

    with tile.TileContext(nc) as tc:
        _body(tc, d_qk, d_qo, d_v, d_wq, d_wk, d_wqo, d_wv, d_wp, d_eye,
              d_zero, d_out)
    nc.compile()
    return nc


def _body(tc, d_qk, d_qo, d_v, d_wq, d_wk, d_wqo, d_wv, d_wp, d_eye,
          d_zero, d_out):
    nc = tc.nc
    _ap = lambda t: t if isinstance(t, bass.AP) else t.ap()
    d_qk, d_qo, d_v, d_wq, d_wk, d_wqo, d_wv, d_wp, d_eye, d_zero, d_out = (
        _ap(t) for t in (d_qk, d_qo, d_v, d_wq, d_wk, d_wqo, d_wv, d_wp,
                         d_eye, d_zero, d_out))
    from contextlib import ExitStack
    ctx = ExitStack()
    xpool = ctx.enter_context(tc.tile_pool(name="xpool", bufs=2))
    wpool = ctx.enter_context(tc.tile_pool(name="wpool", bufs=2))
    spool = ctx.enter_context(tc.tile_pool(name="spool", bufs=2))
    persist = ctx.enter_context(tc.tile_pool(name="persist", bufs=1))
    apool = ctx.enter_context(tc.tile_pool(name="apool", bufs=2))
    rpool = ctx.enter_context(tc.tile_pool(name="rpool", bufs=2))
    tpool = ctx.enter_context(tc.tile_pool(name="tpool", bufs=2))
    opool = ctx.enter_context(tc.tile_pool(name="opool", bufs=2))
    psS = ctx.enter_context(tc.tile_pool(name="psS", bufs=2, space="PSUM"))
    psO = ctx.enter_context(tc.tile_pool(name="psO", bufs=1, space="PSUM"))
    psP = ctx.enter_context(tc.tile_pool(name="psP", bufs=1, space="PSUM"))

    # ---- persistent tensors -------------------------------------------------
    # kblk[k, co, h, mt, M]: bf16 blockdiag stationary per (co,h,mt):
    #   rows 0-63 x cols 0-63 = k[d, mA], rows 64-127 x cols 64-127 = k[d, mB]
    kblk = persist.tile([P, CT, 2, NT, P], BF16, name="kblk")
    # q duplicated to both partition halves; per-co overwritten by qo after
    # the (co, att=0, *) heads have been emitted (emission order = dep order)
    qdup = persist.tile([P, CT, 2, N], BF16, name="qdup")
    # V[m, head, d] bf16 with a ones column at d=64 (softmax denominator)
    vsb = persist.tile([P, NT, H, HD + 1], BF16, name="vsb")
    onorm = persist.tile([P, NT, C], BF16, name="onorm")   # O[n, c]
    onormT = persist.tile([P, CT, N], BF16, name="onormT")  # O^T[c, n]
    eye = persist.tile([P, P], BF16, name="eye")

    # ---- kblk off-diagonal zeros via DMA (sync queue; co=0 first) ----------
    def zdma(co, h):
        for parts, coff in ((slice(0, 64), 64), (slice(64, P), 0)):
            base = kblk[parts, co, h, 0, coff:coff + 64]
            dst = bass.AP(tensor=base.tensor, offset=base.offset,
                          ap=[list(base.ap[0]), [P, NT], [1, 64]])
            src = bass.AP(tensor=d_zero.tensor, offset=0,
                          ap=[[0, 64], [64, NT], [1, 64]])
            nc.sync.dma_start(dst, src)

    # ---- load inputs (spread across engine DMA queues) ---------------------
    xqk = xpool.tile([P, CT, 2, N], FP8, tag="x8", name="xqk")
    xqo = xpool.tile([P, CT, 2, N], FP8, tag="x8", name="xqo")
    xv = xpool.tile([P, CT, 2, N], FP8, tag="x8b", name="xv", bufs=1)
    rearr_x = lambda d: d.rearrange("(t p) v n -> p t v n", p=P)
    nc.sync.dma_start(xqk[:], rearr_x(d_qk))
    nc.sync.dma_start(eye[:], d_eye)
    zdma(0, 0)
    zdma(0, 1)
    nc.sync.dma_start(xv[:], rearr_x(d_v))
    nc.scalar.dma_start(xqo[:], rearr_x(d_qo))

    # ---- phase 1: projections (split-fp8 DoubleRow), co-granular ----------
    def make_proj(d_w, srcx, sink, name, eng=None):
        wsb = wpool.tile([P, CT, 2, C], FP8, tag="w8", name=name, bufs=3)
        (eng or nc.gpsimd).dma_start(
            wsb[:], d_w.rearrange("(t p) v co -> p t v co", p=P))
        terms = [(0, 0), (1, 0), (0, 1)]  # (w-slot, x-slot)

        def co_fn(co):
            ps = psP.tile([P, N], FP32, tag="p", name="ps_qkv")
            cosl = slice(co * P, (co + 1) * P)
            for ch in range(2):
                nsl = slice(ch * 512, (ch + 1) * 512)
                for ti, (wv_, xv_) in enumerate(terms):
                    for j in range(3):
                        nc.tensor.matmul(
                            ps[:, nsl],
                            wsb[:, 2 * j:2 * j + 2, wv_, cosl],
                            srcx[:, 2 * j:2 * j + 2, xv_, nsl],
                            start=(ti == 0 and j == 0),
                            stop=(ti == 2 and j == 2), perf_mode=DR)
            sink(co, ps)
        return co_fn

    def dup_sink(co, ps):
        stg = spool.tile([P, N], BF16, tag="stg", name="qstg")
        nc.vector.tensor_copy(stg[:], ps[:])
        nc.gpsimd.dma_start(qdup[0:64, co, 0, :], stg[0:64, :])
        nc.gpsimd.dma_start(qdup[64:P, co, 0, :], stg[0:64, :])
        nc.gpsimd.dma_start(qdup[0:64, co, 1, :], stg[64:P, :])
        nc.gpsimd.dma_start(qdup[64:P, co, 1, :], stg[64:P, :])

    def k_sink(co, ps):
        stg = spool.tile([P, N], BF16, tag="stg", name="kstg")
        nc.vector.tensor_copy(stg[:], ps[:])

        def stg_ap(prt, half):
            s = stg[prt, half * 64:half * 64 + 64]
            return bass.AP(tensor=s.tensor, offset=s.offset,
                           ap=[list(s.ap[0]), [128, NT], [1, 64]])
        nc.gpsimd.dma_start(kblk[0:64, co, 0, :, 0:64], stg_ap(slice(0, 64), 0))
        nc.gpsimd.dma_start(kblk[64:P, co, 0, :, 64:P], stg_ap(slice(0, 64), 1))
        nc.gpsimd.dma_start(kblk[0:64, co, 1, :, 0:64], stg_ap(slice(64, P), 0))
        nc.gpsimd.dma_start(kblk[64:P, co, 1, :, 64:P], stg_ap(slice(64, P), 1))

    def make_vproj():
        wsb = wpool.tile([P, CT, 2, C], FP8, tag="w8", name="wv8", bufs=3)
        nc.gpsimd.dma_start(wsb[:],
                            d_wv.rearrange("(t p) v co -> p t v co", p=P))

        def mt_fn(mt):
            ps = psP.tile([P, N], FP32, tag="p", name="ps_v")
            msl = slice(mt * P, (mt + 1) * P)
            for base, wd in ((0, 512), (512, 256)):
                terms = [(0, 0), (0, 1), (1, 0)]  # (x-slot, w-slot)
                for ti, (xv_, wv_) in enumerate(terms):
                    for j in range(3):
                        nc.tensor.matmul(
                            ps[:, base:base + wd],
                            xv[:, 2 * j:2 * j + 2, xv_, msl],
                            wsb[:, 2 * j:2 * j + 2, wv_, base:base + wd],
                            start=(ti == 0 and j == 0),
                            stop=(ti == 2 and j == 2), perf_mode=DR)
            nc.vector.tensor_scalar_mul(
                vsb[:, mt, :, 0:HD],
                ps[:, 0:C].rearrange("p (h d) -> p h d", h=H),
                1.0 / WSCALE)
        return mt_fn

    # ---- phase 2: attention, pipelined at (p, att, head) granularity --------
    # AV slices of the previous head are woven into this head's S/exp steps
    # front-loaded (2,2,1,1,1,1 over weave steps 0-5) so o frees up with two
    # steps of margin before the next head's o allocation (psO has 1 buf).
    AV_SCHED = {0: (0, 1), 1: (2, 3), 2: (4,), 3: (5,), 4: (6,), 5: (7,)}

    def emit_av_slice(pend, nt):
        pp, patt, ph, a, o = pend
        hh = 2 * pp + ph
        ntsl = slice(nt * P, (nt + 1) * P)
        for mt in range(NT):
            nc.tensor.matmul(
                o[:, nt, 0:HD + 1], a[:, mt, ntsl],
                vsb[:, mt, hh, :],
                start=(mt == 0), stop=(mt == NT - 1),
                skip_group_check=True)

    def emit_norm(pend):
        pp, patt, ph, a, o = pend
        hh = 2 * pp + ph
        r = rpool.tile([P, NT, 1], FP32, tag="r", name="r_den")
        nc.vector.reciprocal(r[:], o[:, :, HD:HD + 1])
        rb = bass.AP(tensor=r.tensor, offset=r[:].offset,
                     ap=[list(r[:].ap[0]), [1, NT], [0, HD]])
        dst = onorm[:, :, hh * HD:(hh + 1) * HD]
        if patt == 0:
            nc.vector.tensor_mul(dst, o[:, :, 0:HD], rb)
        else:
            t = tpool.tile([P, NT, HD], BF16, tag="t", name="t_norm")
            nc.vector.tensor_mul(t[:], o[:, :, 0:HD], rb)
            nc.vector.tensor_add(dst, dst, t[:])

    def emit_transpose(p):
        trp = psP.tile([P, N], BF16, tag="p", name="tr")
        for nt in range(NT):
            nc.tensor.transpose(trp[:, nt * P:(nt + 1) * P],
                                onorm[:, nt, p * P:(p + 1) * P], eye[:])
        nc.vector.tensor_copy(onormT[:, p, :], trp[:])

    def emit_head(idx, p, att, h, pend, sched):
        sgn = 1.0 if att == 0 else -1.0
        a = apool.tile([P, NT, N], BF16, tag="a", name="a_att")
        if pend is not None:
            o = psO.tile([P, NT, P], FP32, tag="o", name="o_av")
            pend = pend + (o,)
        for mt in range(NT):
            s = psS.tile([P, N], FP32, tag="s", name="s_att")
            for ch in range(2):
                nsl = slice(ch * 512, (ch + 1) * 512)
                nc.tensor.matmul(
                    s[:, nsl], kblk[:, p, h, mt, :], qdup[:, p, h, nsl],
                    start=True, stop=True)
            if mt in DVE_MTS:
                nc.vector.tensor_scalar(
                    a[:, mt, :].bitcast(I16), s[:],
                    sgn * EXP_A, EXP_B, MUL, ADD)
            else:
                nc.scalar.activation(a[:, mt, :], s[:], EXP,
                                     scale=sgn * EXPSC)
            if pend is not None:
                for nt in AV_SCHED.get(mt, ()):
                    emit_av_slice(pend, nt)
                if mt == 5:
                    emit_norm(pend)
                    if pend[1] == 1 and pend[2] == 1:
                        emit_transpose(pend[0])
            for fn in sched.get((idx, mt), ()):
                fn()
        return a

    # weight loads: wq on the scalar queue, the rest on gpsimd
    kco = make_proj(d_wk, xqk, k_sink, "wk8")
    qco = make_proj(d_wq, xqk, dup_sink, "wq8", eng=nc.scalar)
    vmt = make_vproj()
    # wqo8 reuses a w8 ring slot (freed when the vmt fillers finish reading
    # wv8) -> its DMA waits on the vmt PE matmuls, so it must not sit ahead
    # of the kco(0) scatters (gpsimd) or the first exps (scalar). Sync queue
    # only has late zero-fills and tail stores behind it.
    qoco = make_proj(d_wqo, xqo, dup_sink, "wqo8", eng=nc.sync)

    # remaining kblk zero fills (co 1-5)
    for co in range(1, CT):
        zdma(co, 0)
        zdma(co, 1)

    # co=0 of Q and K inline so the exp stream starts as early as possible
    kco(0)
    qco(0)
    nc.vector.memset(vsb[:, :, :, HD:HD + 1], 1.0)
    for mt in range(4):
        vmt(mt)
    wp = wpool.tile([P, CT, C], BF16, tag="wf", name="wp", bufs=1)
    nc.gpsimd.dma_start(wp[:], d_wp.rearrange("(t p) co -> p t co", p=P))

    heads = [(0, 0, 0), (0, 0, 1), (4, 0, 0), (4, 0, 1), (1, 0, 0),
             (1, 0, 1), (0, 1, 0), (0, 1, 1), (4, 1, 0), (4, 1, 1),
             (2, 0, 0), (2, 0, 1), (1, 1, 0), (1, 1, 1), (3, 0, 0),
             (3, 0, 1), (2, 1, 0), (2, 1, 1), (5, 0, 0), (5, 0, 1),
             (3, 1, 0), (3, 1, 1), (5, 1, 0), (5, 1, 1)]
    # Explicit filler schedule, (head idx, weave mt) -> work, chosen so every
    # projection is emitted before its first reader and the ~38us of filler
    # PE work spreads across the span (PE would otherwise starve while the
    # exp engines drain the s-ring). qoco(p) overwrites qdup[:, p] and so
    # must land after head (p,0,1) and before (p,1,0).
    sched = {
        (0, 1): [lambda: vmt(4)], (0, 3): [lambda: vmt(5)],
        (0, 5): [lambda: vmt(6)], (0, 7): [lambda: vmt(7)],
        (1, 3): [lambda: qco(4)], (1, 6): [lambda: kco(4)],
        (2, 3): [lambda: qco(1)], (2, 6): [lambda: qoco(0)],
        (3, 3): [lambda: kco(1)],
        (4, 3): [lambda: qco(2)],
        (5, 3): [lambda: qoco(4)],
        (6, 3): [lambda: kco(2)],
        (7, 3): [lambda: qco(3)],
        (8, 3): [lambda: qoco(1)],
        (9, 3): [lambda: kco(3)],
        (10, 3): [lambda: qco(5)],
        (12, 3): [lambda: kco(5)],
        (13, 3): [lambda: qoco(2)],
        (17, 3): [lambda: qoco(3)],
        (20, 3): [lambda: qoco(5)],
    }

    pend = None
    for idx, (p, att, h) in enumerate(heads):
        a = emit_head(idx, p, att, h, pend, sched)
        pend = (p, att, h, a)
    o = psO.tile([P, NT, P], FP32, tag="o", name="o_av")
    pend = pend + (o,)
    for nt in range(NT):
        emit_av_slice(pend, nt)
    emit_norm(pend)
    emit_transpose(pend[0])

    # ---- phase 3: output projection (alternating psum slots) ---------------
    def proj_nt(nt, ps):
        for base, wd in ((0, 512), (512, 256)):
            for ct in range(CT):
                nc.tensor.matmul(
                    ps[:, base:base + wd],
                    onormT[:, ct, nt * P:(nt + 1) * P],
                    wp[:, ct, base:base + wd],
                    start=(ct == 0), stop=(ct == CT - 1))
        osb = opool.tile([P, C], FP32, tag="out", name="osb")
        nc.vector.tensor_copy(osb[:], ps[:, 0:C])
        nc.sync.dma_start(d_out[nt * P:(nt + 1) * P, :], osb[:])

    for nt in range(NT):
        pool = psP if nt % 2 == 0 else psS
        tag = "p" if nt % 2 == 0 else "s"
        ps = pool.tile([P, N], FP32, tag=tag, name="ps_proj")
        proj_nt(nt, ps)

    ctx.close()


_NC = None


def _get_nc():
    global _NC
    if _NC is None:
        _NC = build_kernel()
    return _NC


def _split8(a):
    """[R, Cc] fp32 -> [R, 2, Cc] fp8: (fp8(a), fp8(a - fp8(a)))."""
    f8 = ml_dtypes.float8_e4m3
    a8 = a.astype(f8)
    da = (a - a8.astype(np.float32)).astype(f8)
    return np.ascontiguousarray(np.stack([a8, da], axis=1))


def prepare_in_maps(x, y, w_qkv, w_proj, b_proj):
    x = np.asarray(x, np.float32)
    y = np.asarray(y, np.float32)
    w_qkv = np.asarray(w_qkv, np.float32)
    w_proj = np.asarray(w_proj, np.float32)

    bf = ml_dtypes.bfloat16
    spw = lambda w: _split8(np.ascontiguousarray(w.T) * WSCALE)
    spx = lambda t: _split8(np.ascontiguousarray(t.T))
    wqo8 = spw(w_qkv[0:C])
    wq8 = spw(w_qkv[C:2 * C])
    wk8 = spw(w_qkv[2 * C:3 * C])
    wv8 = spw(w_qkv[3 * C:4 * C])
    wp16 = np.ascontiguousarray(w_proj.T).astype(bf)
    eye16 = np.eye(P, dtype=bf)
    zeros16 = np.zeros((1, 8192), dtype=bf)

    in_maps = []
    for i in range(8):
        b = i % 4
        isx = i < 4
        t_qk = x[b] if isx else y[b]
        t_qo = y[b] if isx else x[b]
        in_maps.append({
            "qk8": spx(t_qk), "qo8": spx(t_qo), "v8": spx(x[b]),
            "wq8": wq8, "wk8": wk8, "wqo8": wqo8, "wv8": wv8,
            "wp16": wp16, "eye16": eye16, "zeros16": zeros16,
        })
    return in_maps


def kernel(x, y, w_qkv, w_proj, b_proj):
    nc = _get_nc()
    in_maps = prepare_in_maps(x, y, w_qkv, w_proj, b_proj)
    res = run_bass_kernel_spmd(nc, in_maps, list(range(8)))
    bpf = np.asarray(b_proj, np.float32)
    out_x = np.stack([res.results[b]["out"] for b in range(4)]) + bpf
    out_y = np.stack([res.results[4 + b]["out"] for b in range(4)]) + bpf
    return out_x.astype(np.float32), out_y.astype(np.float32)


if __name__ == "__main__":
    rng = np.random.default_rng(0)
    ins = {
        "x": rng.standard_normal((B, N, C), dtype=np.float32),
        "y": rng.standard_normal((B, N, C), dtype=np.float32),
        "w_qkv": (rng.standard_normal((4 * C, C)) * 0.02).astype(np.float32),
        "w_proj": (rng.standard_normal((C, C)) * 0.02).astype(np.float32),
        "b_proj": (rng.standard_normal(C) * 0.02).astype(np.float32),
    }
    ox, oy = kernel(**ins)
    print(ox.shape, oy.shape, ox.dtype)


# revision 11
# speedup vs baseline: 1.3002x; 1.3002x over previous
"""Trainium2 Bass kernel for the dual-stream "DifAttention" block — v5.

Partitioning (unchanged): 8 independent (batch, stream) units, one per core,
SPMD, no collectives:
    x-core b: t_qk=x[b], t_v=x[b], t_qo=y[b]
    y-core b: t_qk=y[b], t_v=x[b], t_qo=x[b]

Design (v5, measured-model driven):

  projections      plain bf16, 12 matmuls per output col-tile (6144 cyc/co).
                   Measured: the v3 split-fp8 3-term DoubleRow form costs
                   9216 cyc/co — DR halves per-term cycles but the 3 terms
                   cost 1.5x bf16. bf16 is both faster AND more accurate.
  S^T = K Q^T      bf16 blockdiag: stationary [128,128] = blockdiag(k[d,mA],
                   k[d,mB]), moving = q duplicated across partition halves.
                   Same 512 cyc per [128m x 512n] as the DR-fp8 variant, but
                   only bf16 cast error (~0.4%) instead of fp8 (~6%).
  exp split        ACT does 6 of 8 m-tiles per (head,att) via native EXP;
                   DVE does 2 of 8 via a Schraudolph bit-trick:
                   a = bitcast_bf16(int16(s*A + B)), A = +-0.125*log2e*128,
                   B = 128*(127 + c), c = -0.0564 (mean-zero centering: no
                   softmax-mass bias between DVE and ACT tiles). This keeps
                   the exp stream off the critical path (PE is the pacer).
  A V              o[n,d] form: stationary = A^T tile [128m x 128n] (FWL
                   pipelines the loads, ~35ns/matmul issue rate), moving =
                   V[m, 64d + ones-col]; denominator lands per-partition.
  out proj         bf16 from onorm^T (PE-transposed via identity matmuls).

Scheduling: explicit (head idx, weave mt) filler schedule spreads the
projection work across the attention span; separate PSUM pools for s-tiles
(2 bufs), o (1), fillers/transposes/out-proj (1) = 8 banks exactly.
"""

import numpy as np
import ml_dtypes

import concourse.bass as bass
import concourse.bacc as bacc
import concourse.tile as tile
from concourse import mybir
from concourse.bass_utils import run_bass_kernel_spmd

P = 128
B, N, C = 4, 1024, 768
H, HD = 12, 64
CT = C // P           # 6 column tiles (= head pairs)
NT = N // P           # 8 sequence tiles
EXPSC = 0.125         # 1/sqrt(hd)

LOG2E = 1.4426950408889634
C_CENTER = -0.056401  # mean-zero Schraudolph centering
EXP_A = EXPSC * LOG2E * 128.0
EXP_B = 128.0 * (127.0 + C_CENTER)
DVE_MTS = (2, 5)      # m-tiles per (head,att) handled by the DVE exp

FP32 = mybir.dt.float32
BF16 = mybir.dt.bfloat16
I16 = mybir.dt.int16
EXP = mybir.ActivationFunctionType.Exp
MUL = mybir.AluOpType.mult
ADD = mybir.AluOpType.add


def build_kernel():
    nc = bacc.Bacc("TRN2", target_bir_lowering=False, debug=False,
                   num_devices=8)

    d_qk = nc.dram_tensor("qk16", [C, N], BF16, kind="ExternalInput")
    d_qo = nc.dram_tensor("qo16", [C, N], BF16, kind="ExternalInput")
    d_v = nc.dram_tensor("v16", [C, N], BF16, kind="ExternalInput")
    d_wq = nc.dram_tensor("wq16", [C, C], BF16, kind="ExternalInput")
    d_wk = nc.dram_tensor("wk16", [C, C], BF16, kind="ExternalInput")
    d_wqo = nc.dram_tensor("wqo16", [C, C], BF16, kind="ExternalInput")
    d_wv = nc.dram_tensor("wv16", [C, C], BF16, kind="ExternalInput")
    d_wp = nc.dram_tensor("wp16", [C, C], BF16, kind="ExternalInput")
    d_eye = nc.dram_tensor("eye16", [P, P], BF16, kind="ExternalInput")
    d_out = nc.dram_tensor("out", [N, C], FP32, kind="ExternalOutput")

    with tile.TileContext(nc) as tc:
        _body(tc, d_qk, d_qo, d_v, d_wq, d_wk, d_wqo, d_wv, d_wp, d_eye,
              d_out)
    nc.compile()
    return nc


def _body(tc, d_qk, d_qo, d_v, d_wq, d_wk, d_wqo, d_wv, d_wp, d_eye, d_out):
    nc = tc.nc
    _ap = lambda t: t if isinstance(t, bass.AP) else t.ap()
    d_qk, d_qo, d_v, d_wq, d_wk, d_wqo, d_wv, d_wp, d_eye, d_out = (
        _ap(t) for t in (d_qk, d_qo, d_v, d_wq, d_wk, d_wqo, d_wv, d_wp,
                         d_eye, d_out))
    from contextlib import ExitStack
    ctx = ExitStack()
    xpool = ctx.enter_context(tc.tile_pool(name="xpool", bufs=2))
    wpool = ctx.enter_context(tc.tile_pool(name="wpool", bufs=2))
    spool = ctx.enter_context(tc.tile_pool(name="spool", bufs=2))
    persist = ctx.enter_context(tc.tile_pool(name="persist", bufs=1))
    apool = ctx.enter_context(tc.tile_pool(name="apool", bufs=2))
    rpool = ctx.enter_context(tc.tile_pool(name="rpool", bufs=2))
    tpool = ctx.enter_context(tc.tile_pool(name="tpool", bufs=2))
    opool = ctx.enter_context(tc.tile_pool(name="opool", bufs=2))
    psS = ctx.enter_context(tc.tile_pool(name="psS", bufs=2, space="PSUM"))
    psO = ctx.enter_context(tc.tile_pool(name="psO", bufs=1, space="PSUM"))
    psP = ctx.enter_context(tc.tile_pool(name="psP", bufs=1, space="PSUM"))

    # ---- persistent tensors -------------------------------------------------
    # kblk[k, co, h, mt, M]: bf16 blockdiag stationary per (co,h,mt):
    #   rows 0-63 x cols 0-63 = k[d, mA], rows 64-127 x cols 64-127 = k[d, mB]
    kblk = persist.tile([P, CT, 2, NT, P], BF16, name="kblk")
    # q duplicated to both partition halves; per-co overwritten by qo after
    # the (co, att=0, *) heads have been emitted (emission order = dep order)
    qdup = persist.tile([P, CT, 2, N], BF16, name="qdup")
    # V[m, head, d] bf16 with a ones column at d=64 (softmax denominator)
    vsb = persist.tile([P, NT, H, HD + 1], BF16, name="vsb")
    onorm = persist.tile([P, NT, C], BF16, name="onorm")   # O[n, c]
    onormT = persist.tile([P, CT, N], BF16, name="onormT")  # O^T[c, n]
    eye = persist.tile([P, P], BF16, name="eye")

    # zero the off-diagonal blocks of kblk on DVE; one memset per
    # (co, partition-half) covers both h (contiguous nt-rows)
    def zmemset(co):
        for parts, coff in ((slice(0, 64), 64), (slice(64, P), 0)):
            base = kblk[parts, co, 0, 0, coff:coff + 64]
            dst = bass.AP(tensor=base.tensor, offset=base.offset,
                          ap=[list(base.ap[0]), [P, 2 * NT], [1, 64]])
            nc.vector.memset(dst, 0.0)

    # ---- load inputs --------------------------------------------------------
    xqk = xpool.tile([P, CT, N], BF16, tag="x16", name="xqk")
    xqo = xpool.tile([P, CT, N], BF16, tag="x16", name="xqo")
    xv = xpool.tile([P, CT, N], BF16, tag="x16b", name="xv", bufs=1)
    rearr_x = lambda d: d.rearrange("(t p) n -> p t n", p=P)
    nc.sync.dma_start(xqk[:], rearr_x(d_qk))
    nc.sync.dma_start(eye[:], d_eye)
    nc.sync.dma_start(xv[:], rearr_x(d_v))
    nc.scalar.dma_start(xqo[:], rearr_x(d_qo))

    # ---- phase 1: projections (bf16), co-granular --------------------------
    def make_proj(d_w, srcx, sink, name, eng=None):
        wsb = wpool.tile([P, CT, C], BF16, tag="w16", name=name, bufs=3)
        (eng or nc.gpsimd).dma_start(
            wsb[:], d_w.rearrange("(t p) co -> p t co", p=P))

        def co_fn(co):
            ps = psP.tile([P, N], FP32, tag="p", name="ps_qkv")
            cosl = slice(co * P, (co + 1) * P)
            for ch in range(2):
                nsl = slice(ch * 512, (ch + 1) * 512)
                for j in range(CT):
                    nc.tensor.matmul(
                        ps[:, nsl], wsb[:, j, cosl], srcx[:, j, nsl],
                        start=(j == 0), stop=(j == CT - 1))
            sink(co, ps)
        return co_fn

    def dup_sink(co, ps):
        stg = spool.tile([P, N], BF16, tag="stg", name="qstg")
        nc.vector.tensor_copy(stg[:], ps[:])
        nc.gpsimd.dma_start(qdup[0:64, co, 0, :], stg[0:64, :])
        nc.gpsimd.dma_start(qdup[64:P, co, 0, :], stg[0:64, :])
        nc.gpsimd.dma_start(qdup[0:64, co, 1, :], stg[64:P, :])
        nc.gpsimd.dma_start(qdup[64:P, co, 1, :], stg[64:P, :])

    def k_sink(co, ps):
        stg = spool.tile([P, N], BF16, tag="stg", name="kstg")
        nc.vector.tensor_copy(stg[:], ps[:])

        def stg_ap(prt, half):
            s = stg[prt, half * 64:half * 64 + 64]
            return bass.AP(tensor=s.tensor, offset=s.offset,
                           ap=[list(s.ap[0]), [128, NT], [1, 64]])
        nc.gpsimd.dma_start(kblk[0:64, co, 0, :, 0:64], stg_ap(slice(0, 64), 0))
        nc.gpsimd.dma_start(kblk[64:P, co, 0, :, 64:P], stg_ap(slice(0, 64), 1))
        nc.gpsimd.dma_start(kblk[0:64, co, 1, :, 0:64], stg_ap(slice(64, P), 0))
        nc.gpsimd.dma_start(kblk[64:P, co, 1, :, 64:P], stg_ap(slice(64, P), 1))

    def make_vproj():
        wsb = wpool.tile([P, CT, C], BF16, tag="w16", name="wv16", bufs=3)
        nc.gpsimd.dma_start(wsb[:],
                            d_wv.rearrange("(t p) co -> p t co", p=P))

        def mt_fn(mt):
            ps = psP.tile([P, N], FP32, tag="p", name="ps_v")
            msl = slice(mt * P, (mt + 1) * P)
            for base, wd in ((0, 512), (512, 256)):
                for j in range(CT):
                    nc.tensor.matmul(
                        ps[:, base:base + wd],
                        xv[:, j, msl], wsb[:, j, base:base + wd],
                        start=(j == 0), stop=(j == CT - 1))
            nc.vector.tensor_copy(
                vsb[:, mt, :, 0:HD],
                ps[:, 0:C].rearrange("p (h d) -> p h d", h=H))
        return mt_fn

    # ---- phase 2: attention, pipelined at (p, att, head) granularity --------
    # AV slices of the previous head are woven front-loaded (2,2,1,1,1,1 over
    # weave steps 0-5) so o frees with two steps of margin before the next
    # head's o allocation (psO has 1 buf).
    AV_SCHED = {0: (0, 1), 1: (2, 3), 2: (4,), 3: (5,), 4: (6,), 5: (7,)}

    def emit_av_slice(pend, nt):
        pp, patt, ph, a, o = pend
        hh = 2 * pp + ph
        ntsl = slice(nt * P, (nt + 1) * P)
        for mt in range(NT):
            nc.tensor.matmul(
                o[:, nt, 0:HD + 1], a[:, mt, ntsl],
                vsb[:, mt, hh, :],
                start=(mt == 0), stop=(mt == NT - 1),
                skip_group_check=True)

    def emit_norm(pend):
        pp, patt, ph, a, o = pend
        hh = 2 * pp + ph
        r = rpool.tile([P, NT, 1], FP32, tag="r", name="r_den")
        nc.vector.reciprocal(r[:], o[:, :, HD:HD + 1])
        rb = bass.AP(tensor=r.tensor, offset=r[:].offset,
                     ap=[list(r[:].ap[0]), [1, NT], [0, HD]])
        dst = onorm[:, :, hh * HD:(hh + 1) * HD]
        if patt == 0:
            nc.vector.tensor_mul(dst, o[:, :, 0:HD], rb)
        else:
            t = tpool.tile([P, NT, HD], BF16, tag="t", name="t_norm")
            nc.vector.tensor_mul(t[:], o[:, :, 0:HD], rb)
            nc.vector.tensor_add(dst, dst, t[:])

    def emit_transpose(p):
        trp = psP.tile([P, N], BF16, tag="p", name="tr")
        for nt in range(NT):
            nc.tensor.transpose(trp[:, nt * P:(nt + 1) * P],
                                onorm[:, nt, p * P:(p + 1) * P], eye[:])
        nc.vector.tensor_copy(onormT[:, p, :], trp[:])

    def emit_head(idx, p, att, h, pend, sched):
        sgn = 1.0 if att == 0 else -1.0
        a = apool.tile([P, NT, N], BF16, tag="a", name="a_att")
        if pend is not None:
            o = psO.tile([P, NT, P], FP32, tag="o", name="o_av")
            pend = pend + (o,)
        for mt in range(NT):
            s = psS.tile([P, N], FP32, tag="s", name="s_att")
            for ch in range(2):
                nsl = slice(ch * 512, (ch + 1) * 512)
                nc.tensor.matmul(
                    s[:, nsl], kblk[:, p, h, mt, :], qdup[:, p, h, nsl],
                    start=True, stop=True)
            if mt in DVE_MTS:
                nc.vector.tensor_scalar(
                    a[:, mt, :].bitcast(I16), s[:],
                    sgn * EXP_A, EXP_B, MUL, ADD)
            else:
                nc.scalar.activation(a[:, mt, :], s[:], EXP,
                                     scale=sgn * EXPSC)
            if pend is not None:
                for nt in AV_SCHED.get(mt, ()):
                    emit_av_slice(pend, nt)
                if mt == 5:
                    emit_norm(pend)
                    if pend[1] == 1 and pend[2] == 1:
                        emit_transpose(pend[0])
            for fn in sched.get((idx, mt), ()):
                fn()
        return a

    # weight loads: wq on the scalar queue, wqo on sync (its w16-ring slot
    # frees only after the vmt fillers, so its DMA must not block the gpsimd
    # scatters or the first exps), the rest on gpsimd
    kco = make_proj(d_wk, xqk, k_sink, "wk16")
    qco = make_proj(d_wq, xqk, dup_sink, "wq16", eng=nc.scalar)
    vmt = make_vproj()
    qoco = make_proj(d_wqo, xqo, dup_sink, "wqo16", eng=nc.sync)

    # co=0 of Q and K inline so the exp stream starts as early as possible
    zmemset(0)
    kco(0)
    qco(0)
    nc.vector.memset(vsb[:, :, :, HD:HD + 1], 1.0)
    for mt in range(4):
        vmt(mt)
    wp = wpool.tile([P, CT, C], BF16, tag="wf", name="wp", bufs=1)
    nc.gpsimd.dma_start(wp[:], d_wp.rearrange("(t p) co -> p t co", p=P))

    heads = [(0, 0, 0), (0, 0, 1), (4, 0, 0), (4, 0, 1), (1, 0, 0),
             (1, 0, 1), (0, 1, 0), (0, 1, 1), (4, 1, 0), (4, 1, 1),
             (2, 0, 0), (2, 0, 1), (1, 1, 0), (1, 1, 1), (3, 0, 0),
             (3, 0, 1), (2, 1, 0), (2, 1, 1), (5, 0, 0), (5, 0, 1),
             (3, 1, 0), (3, 1, 1), (5, 1, 0), (5, 1, 1)]
    # Explicit filler schedule, (head idx, weave mt) -> work, chosen so every
    # projection is emitted before its first reader and the filler PE work
    # spreads across the span. qoco(p) overwrites qdup[:, p] and so must land
    # after head (p,0,1) and before (p,1,0).
    sched = {
        (0, 1): [lambda: vmt(4)], (0, 3): [lambda: vmt(5)],
        (0, 5): [lambda: vmt(6)], (0, 7): [lambda: vmt(7)],
        (1, 1): [lambda: zmemset(4)],
        (1, 3): [lambda: qco(4)], (1, 6): [lambda: kco(4)],
        (2, 1): [lambda: zmemset(1)],
        (2, 3): [lambda: qco(1)], (2, 6): [lambda: qoco(0)],
        (3, 3): [lambda: kco(1)],
        (4, 3): [lambda: qco(2)],
        (5, 3): [lambda: qoco(4)], (5, 6): [lambda: zmemset(2)],
        (6, 3): [lambda: kco(2)],
        (7, 3): [lambda: qco(3)],
        (8, 3): [lambda: qoco(1)], (8, 6): [lambda: zmemset(3)],
        (9, 3): [lambda: kco(3)],
        (10, 3): [lambda: qco(5)],
        (11, 3): [lambda: zmemset(5)],
        (12, 3): [lambda: kco(5)],
        (13, 3): [lambda: qoco(2)],
        (17, 3): [lambda: qoco(3)],
        (20, 3): [lambda: qoco(5)],
    }

    pend = None
    for idx, (p, att, h) in enumerate(heads):
        a = emit_head(idx, p, att, h, pend, sched)
        pend = (p, att, h, a)
    o = psO.tile([P, NT, P], FP32, tag="o", name="o_av")
    pend = pend + (o,)
    for nt in range(NT):
        emit_av_slice(pend, nt)
    emit_norm(pend)
    emit_transpose(pend[0])

    # ---- phase 3: output projection (alternating psum slots) ---------------
    def proj_nt(nt, ps):
        for base, wd in ((0, 512), (512, 256)):
            for ct in range(CT):
                nc.tensor.matmul(
                    ps[:, base:base + wd],
                    onormT[:, ct, nt * P:(nt + 1) * P],
                    wp[:, ct, base:base + wd],
                    start=(ct == 0), stop=(ct == CT - 1))
        osb = opool.tile([P, C], FP32, tag="out", name="osb")
        nc.vector.tensor_copy(osb[:], ps[:, 0:C])
        nc.sync.dma_start(d_out[nt * P:(nt + 1) * P, :], osb[:])

    for nt in range(NT):
        pool = psP if nt % 2 == 0 else psS
        tag = "p" if nt % 2 == 0 else "s"
        ps = pool.tile([P, N], FP32, tag=tag, name="ps_proj")
        proj_nt(nt, ps)

    ctx.close()


_NC = None


def _get_nc():
    global _NC
    if _NC is None:
        _NC = build_kernel()
    return _NC


def prepare_in_maps(x, y, w_qkv, w_proj, b_proj):
    x = np.asarray(x, np.float32)
    y = np.asarray(y, np.float32)
    w_qkv = np.asarray(w_qkv, np.float32)
    w_proj = np.asarray(w_proj, np.float32)

    bf = ml_dtypes.bfloat16
    tb = lambda a: np.ascontiguousarray(a.T).astype(bf)
    wqo16 = tb(w_qkv[0:C])
    wq16 = tb(w_qkv[C:2 * C])
    wk16 = tb(w_qkv[2 * C:3 * C])
    wv16 = tb(w_qkv[3 * C:4 * C])
    wp16 = tb(w_proj)
    eye16 = np.eye(P, dtype=bf)

    in_maps = []
    for i in range(8):
        b = i % 4
        isx = i < 4
        t_qk = x[b] if isx else y[b]
        t_qo = y[b] if isx else x[b]
        in_maps.append({
            "qk16": tb(t_qk), "qo16": tb(t_qo), "v16": tb(x[b]),
            "wq16": wq16, "wk16": wk16, "wqo16": wqo16, "wv16": wv16,
            "wp16": wp16, "eye16": eye16,
        })
    return in_maps


def kernel(x, y, w_qkv, w_proj, b_proj):
    nc = _get_nc()
    in_maps = prepare_in_maps(x, y, w_qkv, w_proj, b_proj)
    res = run_bass_kernel_spmd(nc, in_maps, list(range(8)))
    bpf = np.asarray(b_proj, np.float32)
    out_x = np.stack([res.results[b]["out"] for b in range(4)]) + bpf
    out_y = np.stack([res.results[4 + b]["out"] for b in range(4)]) + bpf
    return out_x.astype(np.float32), out_y.astype(np.float32)


if __name__ == "__main__":
    rng = np.random.default_rng(0)
    ins = {
        "x": rng.standard_normal((B, N, C), dtype=np.float32),
        "y": rng.standard_normal((B, N, C), dtype=np.float32),
        "w_qkv": (rng.standard_normal((4 * C, C)) * 0.02).astype(np.float32),
        "w_proj": (rng.standard_normal((C, C)) * 0.02).astype(np.float32),
        "b_proj": (rng.standard_normal(C) * 0.02).astype(np.float32),
    }
    ox, oy = kernel(**ins)
    print(ox.shape, oy.shape, ox.dtype)


# revision 21
# speedup vs baseline: 1.3117x; 1.0088x over previous
"""Trainium2 Bass kernel for the dual-stream "DifAttention" block — v5.

Partitioning (unchanged): 8 independent (batch, stream) units, one per core,
SPMD, no collectives:
    x-core b: t_qk=x[b], t_v=x[b], t_qo=y[b]
    y-core b: t_qk=y[b], t_v=x[b], t_qo=x[b]

Design (v5, measured-model driven):

  projections      plain bf16, 12 matmuls per output col-tile (6144 cyc/co).
                   Measured: the v3 split-fp8 3-term DoubleRow form costs
                   9216 cyc/co — DR halves per-term cycles but the 3 terms
                   cost 1.5x bf16. bf16 is both faster AND more accurate.
  S^T = K Q^T      bf16 blockdiag: stationary [128,128] = blockdiag(k[d,mA],
                   k[d,mB]), moving = q duplicated across partition halves.
                   Same 512 cyc per [128m x 512n] as the DR-fp8 variant, but
                   only bf16 cast error (~0.4%) instead of fp8 (~6%).
  exp split        ACT does 6 of 8 m-tiles per (head,att) via native EXP;
                   DVE does 2 of 8 via a Schraudolph bit-trick:
                   a = bitcast_bf16(int16(s*A + B)), A = +-0.125*log2e*128,
                   B = 128*(127 + c), c = -0.0564 (mean-zero centering: no
                   softmax-mass bias between DVE and ACT tiles). This keeps
                   the exp stream off the critical path (PE is the pacer).
  A V              o[n,d] form: stationary = A^T tile [128m x 128n] (FWL
                   pipelines the loads, ~35ns/matmul issue rate), moving =
                   V[m, 64d + ones-col]; denominator lands per-partition.
  out proj         bf16 from onorm^T (PE-transposed via identity matmuls).

Scheduling: explicit (head idx, weave mt) filler schedule spreads the
projection work across the attention span; separate PSUM pools for s-tiles
(2 bufs), o (1), fillers/transposes/out-proj (1) = 8 banks exactly.
"""

import numpy as np
import ml_dtypes

import concourse.bass as bass
import concourse.bacc as bacc
import concourse.tile as tile
from concourse import mybir
from concourse.bass_utils import run_bass_kernel_spmd

P = 128
B, N, C = 4, 1024, 768
H, HD = 12, 64
CT = C // P           # 6 column tiles (= head pairs)
NT = N // P           # 8 sequence tiles
EXPSC = 0.125         # 1/sqrt(hd)

LOG2E = 1.4426950408889634
C_CENTER = -0.056401  # mean-zero Schraudolph centering
EXP_A = EXPSC * LOG2E * 128.0
EXP_B = 128.0 * (127.0 + C_CENTER)
DVE_MTS = (1, 4, 6)   # m-tiles per (head,att) handled by the DVE exp

FP32 = mybir.dt.float32
BF16 = mybir.dt.bfloat16
I16 = mybir.dt.int16
EXP = mybir.ActivationFunctionType.Exp
MUL = mybir.AluOpType.mult
ADD = mybir.AluOpType.add


def build_kernel():
    nc = bacc.Bacc("TRN2", target_bir_lowering=False, debug=False,
                   num_devices=8)

    d_qk = nc.dram_tensor("qk16", [C, N], BF16, kind="ExternalInput")
    d_qo = nc.dram_tensor("qo16", [C, N], BF16, kind="ExternalInput")
    d_v = nc.dram_tensor("v16", [C, N], BF16, kind="ExternalInput")
    d_wq = nc.dram_tensor("wq16", [C, C], BF16, kind="ExternalInput")
    d_wk = nc.dram_tensor("wk16", [C, C], BF16, kind="ExternalInput")
    d_wqo = nc.dram_tensor("wqo16", [C, C], BF16, kind="ExternalInput")
    d_wv = nc.dram_tensor("wv16", [C, C], BF16, kind="ExternalInput")
    d_wp = nc.dram_tensor("wp16", [C, C], BF16, kind="ExternalInput")
    d_eye = nc.dram_tensor("eye16", [P, P], BF16, kind="ExternalInput")
    d_out = nc.dram_tensor("out", [N, C], FP32, kind="ExternalOutput")

    with tile.TileContext(nc) as tc:
        _body(tc, d_qk, d_qo, d_v, d_wq, d_wk, d_wqo, d_wv, d_wp, d_eye,
              d_out)
    nc.compile()
    return nc


def _body(tc, d_qk, d_qo, d_v, d_wq, d_wk, d_wqo, d_wv, d_wp, d_eye, d_out):
    nc = tc.nc
    _ap = lambda t: t if isinstance(t, bass.AP) else t.ap()
    d_qk, d_qo, d_v, d_wq, d_wk, d_wqo, d_wv, d_wp, d_eye, d_out = (
        _ap(t) for t in (d_qk, d_qo, d_v, d_wq, d_wk, d_wqo, d_wv, d_wp,
                         d_eye, d_out))
    from contextlib import ExitStack
    ctx = ExitStack()
    xpool = ctx.enter_context(tc.tile_pool(name="xpool", bufs=2))
    wpool = ctx.enter_context(tc.tile_pool(name="wpool", bufs=2))
    spool = ctx.enter_context(tc.tile_pool(name="spool", bufs=2))
    persist = ctx.enter_context(tc.tile_pool(name="persist", bufs=1))
    apool = ctx.enter_context(tc.tile_pool(name="apool", bufs=2))
    rpool = ctx.enter_context(tc.tile_pool(name="rpool", bufs=2))
    tpool = ctx.enter_context(tc.tile_pool(name="tpool", bufs=2))
    opool = ctx.enter_context(tc.tile_pool(name="opool", bufs=2))
    psS = ctx.enter_context(tc.tile_pool(name="psS", bufs=2, space="PSUM"))
    psO = ctx.enter_context(tc.tile_pool(name="psO", bufs=1, space="PSUM"))
    psP = ctx.enter_context(tc.tile_pool(name="psP", bufs=1, space="PSUM"))

    # ---- persistent tensors -------------------------------------------------
    # kblk[k, co, h, mt, M]: bf16 blockdiag stationary per (co,h,mt):
    #   rows 0-63 x cols 0-63 = k[d, mA], rows 64-127 x cols 64-127 = k[d, mB]
    kblk = persist.tile([P, CT, 2, NT, P], BF16, name="kblk")
    # q duplicated to both partition halves; per-co overwritten by qo after
    # the (co, att=0, *) heads have been emitted (emission order = dep order)
    qdup = persist.tile([P, CT, 2, N], BF16, name="qdup")
    # V[m, head, d] bf16 with a ones column at d=64 (softmax denominator)
    vsb = persist.tile([P, NT, H, HD + 1], BF16, name="vsb")
    onorm = persist.tile([P, NT, C], BF16, name="onorm")   # O[n, c]
    onormT = persist.tile([P, CT, N], BF16, name="onormT")  # O^T[c, n]
    eye = persist.tile([P, P], BF16, name="eye")

    # zero the off-diagonal blocks of kblk on DVE; one memset per
    # (co, partition-half) covers both h (contiguous nt-rows)
    def zmemset(co):
        for parts, coff in ((slice(0, 64), 64), (slice(64, P), 0)):
            base = kblk[parts, co, 0, 0, coff:coff + 64]
            dst = bass.AP(tensor=base.tensor, offset=base.offset,
                          ap=[list(base.ap[0]), [P, 2 * NT], [1, 64]])
            nc.vector.memset(dst, 0.0)

    # ---- load inputs --------------------------------------------------------
    # Per-c-tile DMAs: each reads 128 CONTIGUOUS source rows (the whole-tensor
    # rearrange walks the source p-major = 768 scattered row reads ~79GB/s;
    # split loads measured ~4x faster)
    def load_split(dst, d_src, eng):
        for t in range(CT):
            eng.dma_start(dst[:, t, :], d_src[t * P:(t + 1) * P, :])

    xqk = xpool.tile([P, CT, N], BF16, tag="x16", name="xqk")
    xqo = xpool.tile([P, CT, N], BF16, tag="x16", name="xqo")
    xv = xpool.tile([P, CT, N], BF16, tag="x16b", name="xv", bufs=1)
    load_split(xqk, d_qk, nc.sync)
    nc.sync.dma_start(eye[:], d_eye)
    load_split(xv, d_v, nc.sync)

    # ---- phase 1: projections (bf16), co-granular --------------------------
    def make_proj(d_w, srcx, sink, name, eng=None):
        wsb = wpool.tile([P, CT, C], BF16, tag="w16", name=name, bufs=3)
        load_split(wsb, d_w, eng or nc.gpsimd)

        def co_fn(co):
            ps = psP.tile([P, N], FP32, tag="p", name="ps_qkv")
            cosl = slice(co * P, (co + 1) * P)
            for ch in range(2):
                nsl = slice(ch * 512, (ch + 1) * 512)
                for j in range(CT):
                    nc.tensor.matmul(
                        ps[:, nsl], wsb[:, j, cosl], srcx[:, j, nsl],
                        start=(j == 0), stop=(j == CT - 1))
            sink(co, ps)
        return co_fn

    def dup_sink(co, ps):
        stg = spool.tile([P, N], BF16, tag="stg", name="qstg")
        nc.vector.tensor_copy(stg[:], ps[:])
        nc.gpsimd.dma_start(qdup[0:64, co, 0, :], stg[0:64, :])
        nc.gpsimd.dma_start(qdup[64:P, co, 0, :], stg[0:64, :])
        nc.gpsimd.dma_start(qdup[0:64, co, 1, :], stg[64:P, :])
        nc.gpsimd.dma_start(qdup[64:P, co, 1, :], stg[64:P, :])

    def k_sink(co, ps):
        stg = spool.tile([P, N], BF16, tag="stg", name="kstg")
        nc.vector.tensor_copy(stg[:], ps[:])

        def stg_ap(prt, half):
            s = stg[prt, half * 64:half * 64 + 64]
            return bass.AP(tensor=s.tensor, offset=s.offset,
                           ap=[list(s.ap[0]), [128, NT], [1, 64]])
        nc.gpsimd.dma_start(kblk[0:64, co, 0, :, 0:64], stg_ap(slice(0, 64), 0))
        nc.gpsimd.dma_start(kblk[64:P, co, 0, :, 64:P], stg_ap(slice(0, 64), 1))
        nc.gpsimd.dma_start(kblk[0:64, co, 1, :, 0:64], stg_ap(slice(64, P), 0))
        nc.gpsimd.dma_start(kblk[64:P, co, 1, :, 64:P], stg_ap(slice(64, P), 1))

    def make_vproj():
        wsb = wpool.tile([P, CT, C], BF16, tag="w16", name="wv16", bufs=3)
        load_split(wsb, d_wv, nc.gpsimd)

        def mt_fn(mt):
            ps = psP.tile([P, N], FP32, tag="p", name="ps_v")
            msl = slice(mt * P, (mt + 1) * P)
            for base, wd in ((0, 512), (512, 256)):
                for j in range(CT):
                    nc.tensor.matmul(
                        ps[:, base:base + wd],
                        xv[:, j, msl], wsb[:, j, base:base + wd],
                        start=(j == 0), stop=(j == CT - 1))
            nc.vector.tensor_copy(
                vsb[:, mt, :, 0:HD],
                ps[:, 0:C].rearrange("p (h d) -> p h d", h=H))
        return mt_fn

    # ---- phase 2: attention, pipelined at (p, att, head) granularity --------
    # AV slices of the previous head are woven front-loaded (2,2,1,1,1,1 over
    # weave steps 0-5) so o frees with two steps of margin before the next
    # head's o allocation (psO has 1 buf).
    AV_SCHED = {0: (0, 1), 1: (2, 3), 2: (4,), 3: (5,), 4: (6,), 5: (7,)}

    def emit_av_slice(pend, nt):
        pp, patt, ph, a, o = pend
        hh = 2 * pp + ph
        ntsl = slice(nt * P, (nt + 1) * P)
        for mt in range(NT):
            nc.tensor.matmul(
                o[:, nt, 0:HD + 1], a[:, mt, ntsl],
                vsb[:, mt, hh, :],
                start=(mt == 0), stop=(mt == NT - 1),
                skip_group_check=True)

    def emit_norm(pend):
        pp, patt, ph, a, o = pend
        hh = 2 * pp + ph
        r = rpool.tile([P, NT, 1], FP32, tag="r", name="r_den")
        nc.vector.reciprocal(r[:], o[:, :, HD:HD + 1])
        rb = bass.AP(tensor=r.tensor, offset=r[:].offset,
                     ap=[list(r[:].ap[0]), [1, NT], [0, HD]])
        dst = onorm[:, :, hh * HD:(hh + 1) * HD]
        if patt == 0:
            nc.vector.tensor_mul(dst, o[:, :, 0:HD], rb)
        else:
            t = tpool.tile([P, NT, HD], BF16, tag="t", name="t_norm")
            nc.vector.tensor_mul(t[:], o[:, :, 0:HD], rb)
            nc.vector.tensor_add(dst, dst, t[:])

    def emit_transpose(p):
        trp = psP.tile([P, N], BF16, tag="p", name="tr")
        for nt in range(NT):
            nc.tensor.transpose(trp[:, nt * P:(nt + 1) * P],
                                onorm[:, nt, p * P:(p + 1) * P], eye[:])
        nc.vector.tensor_copy(onormT[:, p, :], trp[:])

    def emit_head(idx, p, att, h, pend, sched):
        sgn = 1.0 if att == 0 else -1.0
        a = apool.tile([P, NT, N], BF16, tag="a", name="a_att")
        if pend is not None:
            o = psO.tile([P, NT, P], FP32, tag="o", name="o_av")
            pend = pend + (o,)
        for mt in range(NT):
            s = psS.tile([P, N], FP32, tag="s", name="s_att")
            for ch in range(2):
                nsl = slice(ch * 512, (ch + 1) * 512)
                nc.tensor.matmul(
                    s[:, nsl], kblk[:, p, h, mt, :], qdup[:, p, h, nsl],
                    start=True, stop=True)
            if mt in DVE_MTS:
                nc.vector.tensor_scalar(
                    a[:, mt, :].bitcast(I16), s[:],
                    sgn * EXP_A, EXP_B, MUL, ADD)
            else:
                nc.scalar.activation(a[:, mt, :], s[:], EXP,
                                     scale=sgn * EXPSC)
            if pend is not None:
                for nt in AV_SCHED.get(mt, ()):
                    emit_av_slice(pend, nt)
                if mt == 5:
                    emit_norm(pend)
                    if pend[1] == 1 and pend[2] == 1:
                        emit_transpose(pend[0])
            for fn in sched.get((idx, mt), ()):
                fn()
        return a

    # weight loads: wq then xqo on the scalar queue, wqo on sync (its
    # w16-ring slot frees only after the vmt fillers, so its DMA must not
    # block the gpsimd scatters or the first exps), the rest on gpsimd
    kco = make_proj(d_wk, xqk, k_sink, "wk16")
    qco = make_proj(d_wq, xqk, dup_sink, "wq16", eng=nc.scalar)
    load_split(xqo, d_qo, nc.scalar)
    vmt = make_vproj()
    qoco = make_proj(d_wqo, xqo, dup_sink, "wqo16", eng=nc.sync)

    # off-diagonal zeros + co=0 of Q and K inline so the exp stream starts
    # as early as possible (the compile-time scheduler hoists the no-dep
    # memsets into the DMA-wait window anyway)
    for co in range(CT):
        zmemset(co)
    kco(0)
    qco(0)
    nc.vector.memset(vsb[:, :, :, HD:HD + 1], 1.0)
    for mt in range(4):
        vmt(mt)
    wp = wpool.tile([P, CT, C], BF16, tag="wf", name="wp", bufs=1)
    load_split(wp, d_wp, nc.gpsimd)

    heads = [(0, 0, 0), (0, 0, 1), (4, 0, 0), (4, 0, 1), (1, 0, 0),
             (1, 0, 1), (0, 1, 0), (0, 1, 1), (4, 1, 0), (4, 1, 1),
             (2, 0, 0), (2, 0, 1), (1, 1, 0), (1, 1, 1), (3, 0, 0),
             (3, 0, 1), (2, 1, 0), (2, 1, 1), (5, 0, 0), (5, 0, 1),
             (3, 1, 0), (3, 1, 1), (5, 1, 0), (5, 1, 1)]
    # Explicit filler schedule, (head idx, weave mt) -> work, chosen so every
    # projection is emitted before its first reader and the filler PE work
    # spreads across the span. qoco(p) overwrites qdup[:, p] and so must land
    # after head (p,0,1) and before (p,1,0).
    sched = {
        (0, 1): [lambda: vmt(4)], (0, 3): [lambda: vmt(5)],
        (0, 5): [lambda: vmt(6)], (0, 7): [lambda: vmt(7)],
        (1, 3): [lambda: qco(4)], (1, 6): [lambda: kco(4)],
        (2, 3): [lambda: qco(1)], (2, 6): [lambda: qoco(0)],
        (3, 3): [lambda: kco(1)],
        (4, 3): [lambda: qco(2)],
        (5, 3): [lambda: qoco(4)],
        (6, 3): [lambda: kco(2)],
        (7, 3): [lambda: qco(3)],
        (8, 3): [lambda: qoco(1)],
        (9, 3): [lambda: kco(3)],
        (10, 3): [lambda: qco(5)],
        (12, 3): [lambda: kco(5)],
        (13, 3): [lambda: qoco(2)],
        (17, 3): [lambda: qoco(3)],
        (20, 3): [lambda: qoco(5)],
    }

    pend = None
    for idx, (p, att, h) in enumerate(heads):
        a = emit_head(idx, p, att, h, pend, sched)
        pend = (p, att, h, a)
    o = psO.tile([P, NT, P], FP32, tag="o", name="o_av")
    pend = pend + (o,)
    for nt in range(NT):
        emit_av_slice(pend, nt)
    emit_norm(pend)
    emit_transpose(pend[0])

    # ---- phase 3: output projection (alternating psum slots) ---------------
    def proj_nt(nt, ps):
        for base, wd in ((0, 512), (512, 256)):
            for ct in range(CT):
                nc.tensor.matmul(
                    ps[:, base:base + wd],
                    onormT[:, ct, nt * P:(nt + 1) * P],
                    wp[:, ct, base:base + wd],
                    start=(ct == 0), stop=(ct == CT - 1))
        osb = opool.tile([P, C], FP32, tag="out", name="osb")
        nc.vector.tensor_copy(osb[:], ps[:, 0:C])
        nc.sync.dma_start(d_out[nt * P:(nt + 1) * P, :], osb[:])

    for nt in range(NT):
        if nt % 2 == 0:
            ps = psP.tile([P, N], FP32, tag="p", name="ps_proj")
        else:
            # borrow the (now idle) o-slot: same 4KB, reshaped flat
            ps = psO.tile([P, NT, P], FP32, tag="o",
                          name="ps_proj2").rearrange("p a b -> p (a b)")
        proj_nt(nt, ps)

    ctx.close()


_NC = None


def _get_nc():
    global _NC
    if _NC is None:
        _NC = build_kernel()
    return _NC


def prepare_in_maps(x, y, w_qkv, w_proj, b_proj):
    x = np.asarray(x, np.float32)
    y = np.asarray(y, np.float32)
    w_qkv = np.asarray(w_qkv, np.float32)
    w_proj = np.asarray(w_proj, np.float32)

    bf = ml_dtypes.bfloat16
    tb = lambda a: np.ascontiguousarray(a.T).astype(bf)
    wqo16 = tb(w_qkv[0:C])
    wq16 = tb(w_qkv[C:2 * C])
    wk16 = tb(w_qkv[2 * C:3 * C])
    wv16 = tb(w_qkv[3 * C:4 * C])
    wp16 = tb(w_proj)
    eye16 = np.eye(P, dtype=bf)

    in_maps = []
    for i in range(8):
        b = i % 4
        isx = i < 4
        t_qk = x[b] if isx else y[b]
        t_qo = y[b] if isx else x[b]
        in_maps.append({
            "qk16": tb(t_qk), "qo16": tb(t_qo), "v16": tb(x[b]),
            "wq16": wq16, "wk16": wk16, "wqo16": wqo16, "wv16": wv16,
            "wp16": wp16, "eye16": eye16,
        })
    return in_maps


def kernel(x, y, w_qkv, w_proj, b_proj):
    nc = _get_nc()
    in_maps = prepare_in_maps(x, y, w_qkv, w_proj, b_proj)
    res = run_bass_kernel_spmd(nc, in_maps, list(range(8)))
    bpf = np.asarray(b_proj, np.float32)
    out_x = np.stack([res.results[b]["out"] for b in range(4)]) + bpf
    out_y = np.stack([res.results[4 + b]["out"] for b in range(4)]) + bpf
    return out_x.astype(np.float32), out_y.astype(np.float32)


if __name__ == "__main__":
    rng = np.random.default_rng(0)
    ins = {
        "x": rng.standard_normal((B, N, C), dtype=np.float32),
        "y": rng.standard_normal((B, N, C), dtype=np.float32),
        "w_qkv": (rng.standard_normal((4 * C, C)) * 0.02).astype(np.float32),
        "w_proj": (rng.standard_normal((C, C)) * 0.02).astype(np.float32),
        "b_proj": (rng.standard_normal(C) * 0.02).astype(np.float32),
    }
    ox, oy = kernel(**ins)
    print(ox.shape, oy.shape, ox.dtype)


# revision 26
# speedup vs baseline: 1.3269x; 1.0116x over previous
"""Trainium2 Bass kernel for the dual-stream "DifAttention" block — v5.

Partitioning (unchanged): 8 independent (batch, stream) units, one per core,
SPMD, no collectives:
    x-core b: t_qk=x[b], t_v=x[b], t_qo=y[b]
    y-core b: t_qk=y[b], t_v=x[b], t_qo=x[b]

Design (v5, measured-model driven):

  projections      plain bf16, 12 matmuls per output col-tile (6144 cyc/co).
                   Measured: the v3 split-fp8 3-term DoubleRow form costs
                   9216 cyc/co — DR halves per-term cycles but the 3 terms
                   cost 1.5x bf16. bf16 is both faster AND more accurate.
  S^T = K Q^T      bf16 blockdiag: stationary [128,128] = blockdiag(k[d,mA],
                   k[d,mB]), moving = q duplicated across partition halves.
                   Same 512 cyc per [128m x 512n] as the DR-fp8 variant, but
                   only bf16 cast error (~0.4%) instead of fp8 (~6%).
  exp split        ACT does 6 of 8 m-tiles per (head,att) via native EXP;
                   DVE does 2 of 8 via a Schraudolph bit-trick:
                   a = bitcast_bf16(int16(s*A + B)), A = +-0.125*log2e*128,
                   B = 128*(127 + c), c = -0.0564 (mean-zero centering: no
                   softmax-mass bias between DVE and ACT tiles). This keeps
                   the exp stream off the critical path (PE is the pacer).
  A V              o[n,d] form: stationary = A^T tile [128m x 128n] (FWL
                   pipelines the loads, ~35ns/matmul issue rate), moving =
                   V[m, 64d + ones-col]; denominator lands per-partition.
  out proj         bf16 from onorm^T (PE-transposed via identity matmuls).

Scheduling: explicit (head idx, weave mt) filler schedule spreads the
projection work across the attention span; separate PSUM pools for s-tiles
(2 bufs), o (1), fillers/transposes/out-proj (1) = 8 banks exactly.
"""

import numpy as np
import ml_dtypes

import concourse.bass as bass
import concourse.bacc as bacc
import concourse.tile as tile
from concourse import mybir
from concourse.bass_utils import run_bass_kernel_spmd

P = 128
B, N, C = 4, 1024, 768
H, HD = 12, 64
CT = C // P           # 6 column tiles (= head pairs)
NT = N // P           # 8 sequence tiles
WSCALE = 32.0         # host pre-scale on Wq/Wk/Wqo (fp8 subnormal avoidance)
EXPSC = 0.125 / (WSCALE * WSCALE)  # q,k both carry the 32x weight scale

LOG2E = 1.4426950408889634
C_CENTER = -0.056401  # mean-zero Schraudolph centering
EXP_A = EXPSC * LOG2E * 128.0
EXP_B = 128.0 * (127.0 + C_CENTER)
DVE_MTS = (1, 4, 6)   # m-tiles per (head,att) handled by the DVE exp

FP32 = mybir.dt.float32
BF16 = mybir.dt.bfloat16
FP8 = mybir.dt.float8e4
I16 = mybir.dt.int16
DR = mybir.MatmulPerfMode.DoubleRow
EXP = mybir.ActivationFunctionType.Exp
MUL = mybir.AluOpType.mult
ADD = mybir.AluOpType.add


def build_kernel():
    nc = bacc.Bacc("TRN2", target_bir_lowering=False, debug=False,
                   num_devices=8)

    d_qk = nc.dram_tensor("qk8", [C, N], FP8, kind="ExternalInput")
    d_qo = nc.dram_tensor("qo8", [C, N], FP8, kind="ExternalInput")
    d_v = nc.dram_tensor("v16", [C, N], BF16, kind="ExternalInput")
    d_wq = nc.dram_tensor("wq8", [C, C], FP8, kind="ExternalInput")
    d_wk = nc.dram_tensor("wk8", [C, C], FP8, kind="ExternalInput")
    d_wqo = nc.dram_tensor("wqo8", [C, C], FP8, kind="ExternalInput")
    d_wv = nc.dram_tensor("wv16", [C, C], BF16, kind="ExternalInput")
    d_wp = nc.dram_tensor("wp16", [C, C], BF16, kind="ExternalInput")
    d_eye = nc.dram_tensor("eye16", [P, P], BF16, kind="ExternalInput")
    d_out = nc.dram_tensor("out", [N, C], FP32, kind="ExternalOutput")

    with tile.TileContext(nc) as tc:
        _body(tc, d_qk, d_qo, d_v, d_wq, d_wk, d_wqo, d_wv, d_wp, d_eye,
              d_out)
    nc.compile()
    return nc


def _body(tc, d_qk, d_qo, d_v, d_wq, d_wk, d_wqo, d_wv, d_wp, d_eye, d_out):
    nc = tc.nc
    _ap = lambda t: t if isinstance(t, bass.AP) else t.ap()
    d_qk, d_qo, d_v, d_wq, d_wk, d_wqo, d_wv, d_wp, d_eye, d_out = (
        _ap(t) for t in (d_qk, d_qo, d_v, d_wq, d_wk, d_wqo, d_wv, d_wp,
                         d_eye, d_out))
    from contextlib import ExitStack
    ctx = ExitStack()
    xpool = ctx.enter_context(tc.tile_pool(name="xpool", bufs=2))
    wpool = ctx.enter_context(tc.tile_pool(name="wpool", bufs=2))
    spool = ctx.enter_context(tc.tile_pool(name="spool", bufs=2))
    persist = ctx.enter_context(tc.tile_pool(name="persist", bufs=1))
    apool = ctx.enter_context(tc.tile_pool(name="apool", bufs=2))
    rpool = ctx.enter_context(tc.tile_pool(name="rpool", bufs=2))
    tpool = ctx.enter_context(tc.tile_pool(name="tpool", bufs=2))
    opool = ctx.enter_context(tc.tile_pool(name="opool", bufs=2))
    psS = ctx.enter_context(tc.tile_pool(name="psS", bufs=2, space="PSUM"))
    psO = ctx.enter_context(tc.tile_pool(name="psO", bufs=1, space="PSUM"))
    psP = ctx.enter_context(tc.tile_pool(name="psP", bufs=1, space="PSUM"))

    # ---- persistent tensors -------------------------------------------------
    # kblk[k, co, h, mt, M]: bf16 blockdiag stationary per (co,h,mt):
    #   rows 0-63 x cols 0-63 = k[d, mA], rows 64-127 x cols 64-127 = k[d, mB]
    kblk = persist.tile([P, CT, 2, NT, P], BF16, name="kblk")
    # q duplicated to both partition halves; per-co overwritten by qo after
    # the (co, att=0, *) heads have been emitted (emission order = dep order)
    qdup = persist.tile([P, CT, 2, N], BF16, name="qdup")
    # V[m, head, d] bf16 with a ones column at d=64 (softmax denominator)
    vsb = persist.tile([P, NT, H, HD + 1], BF16, name="vsb")
    onorm = persist.tile([P, NT, C], BF16, name="onorm")   # O[n, c]
    onormT = persist.tile([P, CT, N], BF16, name="onormT")  # O^T[c, n]
    eye = persist.tile([P, P], BF16, name="eye")

    # zero the off-diagonal blocks of kblk on DVE; one memset per
    # (co, partition-half) covers both h (contiguous nt-rows)
    def zmemset(co):
        for parts, coff in ((slice(0, 64), 64), (slice(64, P), 0)):
            base = kblk[parts, co, 0, 0, coff:coff + 64]
            dst = bass.AP(tensor=base.tensor, offset=base.offset,
                          ap=[list(base.ap[0]), [P, 2 * NT], [1, 64]])
            nc.vector.memset(dst, 0.0)

    # ---- load inputs --------------------------------------------------------
    # Per-c-tile DMAs: each reads 128 CONTIGUOUS source rows (the whole-tensor
    # rearrange walks the source p-major = 768 scattered row reads ~79GB/s;
    # split loads measured ~4x faster)
    def load_split(dst, d_src, eng):
        for t in range(CT):
            eng.dma_start(dst[:, t, :], d_src[t * P:(t + 1) * P, :])

    xqk = xpool.tile([P, CT, N], FP8, tag="x8", name="xqk")
    xqo = xpool.tile([P, CT, N], FP8, tag="x8", name="xqo")
    xv = xpool.tile([P, CT, N], BF16, tag="x16b", name="xv", bufs=1)
    load_split(xqk, d_qk, nc.sync)
    nc.sync.dma_start(eye[:], d_eye)
    load_split(xv, d_v, nc.sync)

    # ---- phase 1: q/k/qo projections (1-term fp8 DoubleRow: 3072 cyc/co,
    # half of bf16; q/k/qo only feed the softmax path, where the fp8 cast
    # noise (~2.5% rms) is damped by the 1/8 logit scale) -------------------
    def make_proj(d_w, srcx, sink, name, eng=None):
        wsb = wpool.tile([P, CT, C], FP8, tag="w16", name=name, bufs=3)
        load_split(wsb, d_w, eng or nc.gpsimd)

        def co_fn(co):
            ps = psP.tile([P, N], FP32, tag="p", name="ps_qkv")
            cosl = slice(co * P, (co + 1) * P)
            for ch in range(2):
                nsl = slice(ch * 512, (ch + 1) * 512)
                for j in range(3):
                    nc.tensor.matmul(
                        ps[:, nsl],
                        wsb[:, 2 * j:2 * j + 2, cosl],
                        srcx[:, 2 * j:2 * j + 2, nsl],
                        start=(j == 0), stop=(j == 2), perf_mode=DR)
            sink(co, ps)
        return co_fn

    def dup_sink(co, ps):
        stg = spool.tile([P, N], BF16, tag="stg", name="qstg")
        nc.vector.tensor_copy(stg[:], ps[:])
        nc.gpsimd.dma_start(qdup[0:64, co, 0, :], stg[0:64, :])
        nc.gpsimd.dma_start(qdup[64:P, co, 0, :], stg[0:64, :])
        nc.gpsimd.dma_start(qdup[0:64, co, 1, :], stg[64:P, :])
        nc.gpsimd.dma_start(qdup[64:P, co, 1, :], stg[64:P, :])

    def k_sink(co, ps):
        stg = spool.tile([P, N], BF16, tag="stg", name="kstg")
        nc.vector.tensor_copy(stg[:], ps[:])

        def stg_ap(prt, half):
            s = stg[prt, half * 64:half * 64 + 64]
            return bass.AP(tensor=s.tensor, offset=s.offset,
                           ap=[list(s.ap[0]), [128, NT], [1, 64]])
        nc.gpsimd.dma_start(kblk[0:64, co, 0, :, 0:64], stg_ap(slice(0, 64), 0))
        nc.gpsimd.dma_start(kblk[64:P, co, 0, :, 64:P], stg_ap(slice(0, 64), 1))
        nc.gpsimd.dma_start(kblk[0:64, co, 1, :, 0:64], stg_ap(slice(64, P), 0))
        nc.gpsimd.dma_start(kblk[64:P, co, 1, :, 64:P], stg_ap(slice(64, P), 1))

    def make_vproj():
        wsb = wpool.tile([P, CT, C], BF16, tag="w16", name="wv16", bufs=3)
        load_split(wsb, d_wv, nc.gpsimd)

        def mt_fn(mt):
            ps = psP.tile([P, N], FP32, tag="p", name="ps_v")
            msl = slice(mt * P, (mt + 1) * P)
            for base, wd in ((0, 512), (512, 256)):
                for j in range(CT):
                    nc.tensor.matmul(
                        ps[:, base:base + wd],
                        xv[:, j, msl], wsb[:, j, base:base + wd],
                        start=(j == 0), stop=(j == CT - 1))
            nc.vector.tensor_copy(
                vsb[:, mt, :, 0:HD],
                ps[:, 0:C].rearrange("p (h d) -> p h d", h=H))
        return mt_fn

    # ---- phase 2: attention, pipelined at (p, att, head) granularity --------
    # AV slices of the previous head are woven front-loaded (2,2,1,1,1,1 over
    # weave steps 0-5) so o frees with two steps of margin before the next
    # head's o allocation (psO has 1 buf).
    AV_SCHED = {0: (0, 1), 1: (2, 3), 2: (4,), 3: (5,), 4: (6,), 5: (7,)}

    def emit_av_slice(pend, nt):
        pp, patt, ph, a, o = pend
        hh = 2 * pp + ph
        ntsl = slice(nt * P, (nt + 1) * P)
        for mt in range(NT):
            nc.tensor.matmul(
                o[:, nt, 0:HD + 1], a[:, mt, ntsl],
                vsb[:, mt, hh, :],
                start=(mt == 0), stop=(mt == NT - 1),
                skip_group_check=True)

    def emit_norm(pend):
        pp, patt, ph, a, o = pend
        hh = 2 * pp + ph
        r = rpool.tile([P, NT, 1], FP32, tag="r", name="r_den")
        nc.vector.reciprocal(r[:], o[:, :, HD:HD + 1])
        rb = bass.AP(tensor=r.tensor, offset=r[:].offset,
                     ap=[list(r[:].ap[0]), [1, NT], [0, HD]])
        dst = onorm[:, :, hh * HD:(hh + 1) * HD]
        if patt == 0:
            nc.vector.tensor_mul(dst, o[:, :, 0:HD], rb)
        else:
            t = tpool.tile([P, NT, HD], BF16, tag="t", name="t_norm")
            nc.vector.tensor_mul(t[:], o[:, :, 0:HD], rb)
            nc.vector.tensor_add(dst, dst, t[:])

    def emit_transpose(p):
        trp = psP.tile([P, N], BF16, tag="p", name="tr")
        for nt in range(NT):
            nc.tensor.transpose(trp[:, nt * P:(nt + 1) * P],
                                onorm[:, nt, p * P:(p + 1) * P], eye[:])
        nc.vector.tensor_copy(onormT[:, p, :], trp[:])

    def emit_head(idx, p, att, h, pend, sched):
        sgn = 1.0 if att == 0 else -1.0
        a = apool.tile([P, NT, N], BF16, tag="a", name="a_att")
        if pend is not None:
            o = psO.tile([P, NT, P], FP32, tag="o", name="o_av")
            pend = pend + (o,)
        for mt in range(NT):
            s = psS.tile([P, N], FP32, tag="s", name="s_att")
            for ch in range(2):
                nsl = slice(ch * 512, (ch + 1) * 512)
                nc.tensor.matmul(
                    s[:, nsl], kblk[:, p, h, mt, :], qdup[:, p, h, nsl],
                    start=True, stop=True)
            if mt in DVE_MTS:
                nc.vector.tensor_scalar(
                    a[:, mt, :].bitcast(I16), s[:],
                    sgn * EXP_A, EXP_B, MUL, ADD)
            else:
                nc.scalar.activation(a[:, mt, :], s[:], EXP,
                                     scale=sgn * EXPSC)
            if pend is not None:
                for nt in AV_SCHED.get(mt, ()):
                    emit_av_slice(pend, nt)
                if mt == 5:
                    emit_norm(pend)
                    if pend[1] == 1 and pend[2] == 1:
                        emit_transpose(pend[0])
            for fn in sched.get((idx, mt), ()):
                fn()
        return a

    # weight loads: wq then xqo on the scalar queue, wqo on sync (its
    # w16-ring slot frees only after the vmt fillers, so its DMA must not
    # block the gpsimd scatters or the first exps), the rest on gpsimd
    kco = make_proj(d_wk, xqk, k_sink, "wk8")
    qco = make_proj(d_wq, xqk, dup_sink, "wq8", eng=nc.scalar)
    load_split(xqo, d_qo, nc.scalar)
    vmt = make_vproj()
    qoco = make_proj(d_wqo, xqo, dup_sink, "wqo8", eng=nc.sync)

    # off-diagonal zeros + co=0 of Q and K inline so the exp stream starts
    # as early as possible (the compile-time scheduler hoists the no-dep
    # memsets into the DMA-wait window anyway)
    for co in range(CT):
        zmemset(co)
    kco(0)
    qco(0)
    nc.vector.memset(vsb[:, :, :, HD:HD + 1], 1.0)
    for mt in range(4):
        vmt(mt)
    wp = wpool.tile([P, CT, C], BF16, tag="wf", name="wp", bufs=1)
    load_split(wp, d_wp, nc.gpsimd)

    heads = [(0, 0, 0), (0, 0, 1), (4, 0, 0), (4, 0, 1), (1, 0, 0),
             (1, 0, 1), (0, 1, 0), (0, 1, 1), (4, 1, 0), (4, 1, 1),
             (2, 0, 0), (2, 0, 1), (1, 1, 0), (1, 1, 1), (3, 0, 0),
             (3, 0, 1), (2, 1, 0), (2, 1, 1), (5, 0, 0), (5, 0, 1),
             (3, 1, 0), (3, 1, 1), (5, 1, 0), (5, 1, 1)]
    # Explicit filler schedule, (head idx, weave mt) -> work, chosen so every
    # projection is emitted before its first reader and the filler PE work
    # spreads across the span. qoco(p) overwrites qdup[:, p] and so must land
    # after head (p,0,1) and before (p,1,0).
    sched = {
        (0, 1): [lambda: vmt(4)], (0, 3): [lambda: vmt(5)],
        (0, 5): [lambda: vmt(6)], (0, 7): [lambda: vmt(7)],
        (1, 3): [lambda: qco(4)], (1, 6): [lambda: kco(4)],
        (2, 3): [lambda: qco(1)], (2, 6): [lambda: qoco(0)],
        (3, 3): [lambda: kco(1)],
        (4, 3): [lambda: qco(2)],
        (5, 3): [lambda: qoco(4)],
        (6, 3): [lambda: kco(2)],
        (7, 3): [lambda: qco(3)],
        (8, 3): [lambda: qoco(1)],
        (9, 3): [lambda: kco(3)],
        (10, 3): [lambda: qco(5)],
        (12, 3): [lambda: kco(5)],
        (13, 3): [lambda: qoco(2)],
        (17, 3): [lambda: qoco(3)],
        (20, 3): [lambda: qoco(5)],
    }

    pend = None
    for idx, (p, att, h) in enumerate(heads):
        a = emit_head(idx, p, att, h, pend, sched)
        pend = (p, att, h, a)
    o = psO.tile([P, NT, P], FP32, tag="o", name="o_av")
    pend = pend + (o,)
    for nt in range(NT):
        emit_av_slice(pend, nt)
    emit_norm(pend)
    emit_transpose(pend[0])

    # ---- phase 3: output projection (alternating psum slots) ---------------
    def proj_nt(nt, ps):
        for base, wd in ((0, 512), (512, 256)):
            for ct in range(CT):
                nc.tensor.matmul(
                    ps[:, base:base + wd],
                    onormT[:, ct, nt * P:(nt + 1) * P],
                    wp[:, ct, base:base + wd],
                    start=(ct == 0), stop=(ct == CT - 1))
        osb = opool.tile([P, C], FP32, tag="out", name="osb")
        nc.vector.tensor_copy(osb[:], ps[:, 0:C])
        nc.sync.dma_start(d_out[nt * P:(nt + 1) * P, :], osb[:])

    for nt in range(NT):
        if nt % 2 == 0:
            ps = psP.tile([P, N], FP32, tag="p", name="ps_proj")
        else:
            # borrow the (now idle) o-slot: same 4KB, reshaped flat
            ps = psO.tile([P, NT, P], FP32, tag="o",
                          name="ps_proj2").rearrange("p a b -> p (a b)")
        proj_nt(nt, ps)

    ctx.close()


_NC = None


def _get_nc():
    global _NC
    if _NC is None:
        _NC = build_kernel()
    return _NC


def prepare_in_maps(x, y, w_qkv, w_proj, b_proj):
    x = np.asarray(x, np.float32)
    y = np.asarray(y, np.float32)
    w_qkv = np.asarray(w_qkv, np.float32)
    w_proj = np.asarray(w_proj, np.float32)

    bf = ml_dtypes.bfloat16
    f8 = ml_dtypes.float8_e4m3
    tb = lambda a: np.ascontiguousarray(a.T).astype(bf)
    t8 = lambda a: np.ascontiguousarray(a.T).astype(f8)
    w8 = lambda a: np.ascontiguousarray(a.T * WSCALE).astype(f8)
    wqo8 = w8(w_qkv[0:C])
    wq8 = w8(w_qkv[C:2 * C])
    wk8 = w8(w_qkv[2 * C:3 * C])
    wv16 = tb(w_qkv[3 * C:4 * C])
    wp16 = tb(w_proj)
    eye16 = np.eye(P, dtype=bf)

    in_maps = []
    for i in range(8):
        b = i % 4
        isx = i < 4
        t_qk = x[b] if isx else y[b]
        t_qo = y[b] if isx else x[b]
        in_maps.append({
            "qk8": t8(t_qk), "qo8": t8(t_qo), "v16": tb(x[b]),
            "wq8": wq8, "wk8": wk8, "wqo8": wqo8, "wv16": wv16,
            "wp16": wp16, "eye16": eye16,
        })
    return in_maps


def kernel(x, y, w_qkv, w_proj, b_proj):
    nc = _get_nc()
    in_maps = prepare_in_maps(x, y, w_qkv, w_proj, b_proj)
    res = run_bass_kernel_spmd(nc, in_maps, list(range(8)))
    bpf = np.asarray(b_proj, np.float32)
    out_x = np.stack([res.results[b]["out"] for b in range(4)]) + bpf
    out_y = np.stack([res.results[4 + b]["out"] for b in range(4)]) + bpf
    return out_x.astype(np.float32), out_y.astype(np.float32)


if __name__ == "__main__":
    rng = np.random.default_rng(0)
    ins = {
        "x": rng.standard_normal((B, N, C), dtype=np.float32),
        "y": rng.standard_normal((B, N, C), dtype=np.float32),
        "w_qkv": (rng.standard_normal((4 * C, C)) * 0.02).astype(np.float32),
        "w_proj": (rng.standard_normal((C, C)) * 0.02).astype(np.float32),
        "b_proj": (rng.standard_normal(C) * 0.02).astype(np.float32),
    }
    ox, oy = kernel(**ins)
    print(ox.shape, oy.shape, ox.dtype)
